# revision 1
# baseline (speedup 1.0000x reference)
"""GPT (4-layer, E=768, H=12, T=1024, B=2, V=50257) forward on 8 trn2 cores.

Sharding:
  - Residual stream x token-sharded: core c owns tokens [c*256,(c+1)*256) of the
    flattened [2048] (batch-major), so cores 0-3 = batch 0, cores 4-7 = batch 1.
  - LN / MLP / residual adds fully token-local.
  - Attention head-sharded within each batch group of 4 cores (3 heads each):
    AllGather (transposed) hidden states per layer, compute q/k/v + attention +
    out-proj partials locally, ReduceScatter back to token shards.
  - lm_head vocab-sharded: final AllGather of lnf(x), each core computes
    [2048, 6284] logit slice (V padded 50257 -> 50272 = 8*6284).
  - All matmuls fp32r (fp32 data, full PE rate at moving-dim >= 256).
"""

import sys
from contextlib import ExitStack
import numpy as np

sys.path.insert(0, "/opt/trn_rl_repo")

import concourse.bass as bass
import concourse.mybir as mybir
import concourse.tile as tile
from concourse import bacc
from concourse.bass_utils import run_bass_kernel_spmd
from concourse.masks import make_identity

L, H, E, T, V = 4, 12, 768, 1024, 50257
B = 2
NC = 8
TS = (B * T) // NC          # 256 tokens per core
VS = 6284                   # vocab slice per core, even (padded V = 50272)
VPAD = VS * NC
HD = 64
NHC = 3                     # heads per core
EPS = 1e-5
SCALE = float(1.0 / np.sqrt(np.float32(E)))
F32 = mybir.dt.float32
F32R = mybir.dt.float32r

_CACHE = {}


def _build_program():
    nc = bacc.Bacc("TRN2", target_bir_lowering=False, debug=False, num_devices=NC)

    # ---- I/O -------------------------------------------------------------
    x0s = nc.dram_tensor("x0s", [TS, E], F32, kind="ExternalInput")
    wqk = nc.dram_tensor("wqk", [L, E, 2 * NHC * HD], F32, kind="ExternalInput")
    bqk = nc.dram_tensor("bqk", [L, 6, 64], F32, kind="ExternalInput")
    wv = nc.dram_tensor("wv", [L, E, 256], F32, kind="ExternalInput")
    bv = nc.dram_tensor("bv", [L, 3, HD], F32, kind="ExternalInput")
    watp = nc.dram_tensor("watp", [L, NHC * HD, E], F32, kind="ExternalInput")
    atpb = nc.dram_tensor("atpb", [L, E], F32, kind="ExternalInput")
    fcw = nc.dram_tensor("fcw", [L, 24, 6, 128, 128], F32, kind="ExternalInput")
    fcb = nc.dram_tensor("fcb", [L, 24, 128], F32, kind="ExternalInput")
    prw = nc.dram_tensor("prw", [L, 4 * E, E], F32, kind="ExternalInput")
    prb = nc.dram_tensor("prb", [L, E], F32, kind="ExternalInput")
    ln1g = nc.dram_tensor("ln1g", [L, E], F32, kind="ExternalInput")
    ln1b = nc.dram_tensor("ln1b", [L, E], F32, kind="ExternalInput")
    ln2g = nc.dram_tensor("ln2g", [L, E], F32, kind="ExternalInput")
    ln2b = nc.dram_tensor("ln2b", [L, E], F32, kind="ExternalInput")
    lnfg = nc.dram_tensor("lnfg", [1, E], F32, kind="ExternalInput")
    lnfb = nc.dram_tensor("lnfb", [1, E], F32, kind="ExternalInput")
    tri = nc.dram_tensor("tri", [128, 128], F32, kind="ExternalInput")
    wteT = nc.dram_tensor("wteT", [E, VS], F32, kind="ExternalInput")
    logits = nc.dram_tensor("logits", [B * T, VS], F32, kind="ExternalOutput")

    g_all = [list(range(NC))]
    g_batch = [[0, 1, 2, 3], [4, 5, 6, 7]]

    def bcast_row(pool, src_ap, n, dtype, w):
        """Replicate a [w] DRAM row across n partitions via broadcast DMA."""
        t = pool.tile([n, w], dtype)
        in_ap = bass.AP(
            tensor=src_ap.tensor,
            offset=src_ap.offset,
            ap=[[0, n]] + [list(p) for p in src_ap.ap],
        )
        if dtype == F32R:
            in_ap = in_ap.bitcast(F32R)
        nc.sync.dma_start(out=t[:], in_=in_ap)
        return t

    with tile.TileContext(nc) as tc, ExitStack() as es:
        const = es.enter_context(tc.tile_pool(name="const", bufs=1))
        xp = es.enter_context(tc.tile_pool(name="xp", bufs=1))
        lnrow = es.enter_context(tc.tile_pool(name="lnrow", bufs=3))
        stat = es.enter_context(tc.tile_pool(name="stat", bufs=4))
        hpool = es.enter_context(tc.tile_pool(name="hpool", bufs=2))
        hTp = es.enter_context(tc.tile_pool(name="hTp", bufs=1))
        dram = es.enter_context(tc.tile_pool(name="dram", bufs=1, space="DRAM"))
        psT = es.enter_context(tc.tile_pool(name="psT", bufs=1, space="PSUM"))


        ident_f = const.tile([128, 128], F32, name="ident_f")
        make_identity(nc, ident_f)
        ident = const.tile([128, 128], F32R, name="ident")
        nc.vector.tensor_copy(ident[:], ident_f[:])
        ones_f = const.tile([128, 1], F32, name="ones_f")
        nc.vector.memset(ones_f, 1.0)
        ones_r = const.tile([128, 1], F32R, name="ones_r")
        nc.vector.tensor_copy(ones_r[:], ones_f[:])
        tri_sb = const.tile([128, 128], F32R)
        nc.sync.dma_start(out=tri_sb[:], in_=tri[:, :].bitcast(F32R))
        eps_sb = const.tile([128, 1], F32)
        nc.vector.memset(eps_sb, EPS)

        # persistent residual stream [256, 768] as two [128, 768] tiles
        x_sb = [xp.tile([128, E], F32, tag=f"x{t}", name=f"x{t}") for t in range(2)]
        for t in range(2):
            nc.sync.dma_start(out=x_sb[t][:], in_=x0s[t * 128:(t + 1) * 128, :])

        # DRAM bounce buffers for collectives
        hT_in = dram.tile([E, TS], F32)
        hT_ag = dram.tile([NC * E, TS], F32)
        rs_in = dram.tile([T, E], mybir.dt.bfloat16)
        rs_out = dram.tile([TS, E], mybir.dt.bfloat16)
        xf_in = dram.tile([E, TS], F32)
        xf_ag = dram.tile([NC * E, TS], F32)

        def layernorm_t(x_ap, g_bc, b_bc, out_tile):
            """LN over free dim (768) of [128, 768] tile -> out (f32r)."""
            stats = stat.tile([128, 3, 6], F32, tag="bn_stats", name="bn_stats_t")
            xr = x_ap.rearrange("p (s d) -> p s d", s=3)
            for s in range(3):
                nc.vector.bn_stats(out=stats[:, s, :], in_=xr[:, s, :])
            mv = stat.tile([128, 2], F32, tag="bn_aggr", name="bn_aggr_t")
            nc.vector.bn_aggr(out=mv[:], in_=stats[:])
            rstd = stat.tile([128, 1], F32, tag="rstd", name="rstd_t")
            nc.scalar.activation(out=rstd[:], in_=mv[:, 1:2],
                                 func=mybir.ActivationFunctionType.Sqrt,
                                 bias=eps_sb[:], scale=1.0)
            nc.vector.reciprocal(out=rstd[:], in_=rstd[:])
            nc.vector.tensor_scalar(out=out_tile[:], in0=x_ap,
                                    scalar1=mv[:, 0:1], scalar2=rstd[:],
                                    op0=mybir.AluOpType.subtract,
                                    op1=mybir.AluOpType.mult)
            nc.vector.tensor_mul(out=out_tile[:], in0=out_tile[:], in1=g_bc[:])
            nc.vector.tensor_add(out=out_tile[:], in0=out_tile[:], in1=b_bc[:])

        def transpose_to(hsrc, dst_tiles, dst_col):
            """hsrc [128,768] f32r -> dst_tiles[k][:, dst_col:dst_col+128]."""
            for k in range(6):
                pt = psT.tile([128, 128], F32R, tag="tr", name="tr")
                nc.tensor.transpose(pt[:], hsrc[:, k * 128:(k + 1) * 128], ident[:])
                dst = dst_tiles[k][:, dst_col:dst_col + 128]
                if k % 2 == 0:
                    nc.vector.tensor_copy(dst, pt[:])
                else:
                    nc.scalar.activation(out=dst, in_=pt[:],
                                         func=mybir.ActivationFunctionType.Copy)

        # Attention AllGather runs over per-batch groups [[0..3],[4..7]], so
        # every core's AG output holds exactly its own batch's 4 rank-blocks
        # at rows [0, 3072) -- the program stays uniform across cores (SPMD).
        es_l = es.enter_context(ExitStack())
        hTbp = es_l.enter_context(tc.tile_pool(name="hTbp", bufs=1))
        wqkp = es_l.enter_context(tc.tile_pool(name="wqkp", bufs=1))
        wvp = es_l.enter_context(tc.tile_pool(name="wvp", bufs=1))
        watpp = es_l.enter_context(tc.tile_pool(name="watpp", bufs=1))
        bias_p = es_l.enter_context(tc.tile_pool(name="bias_p", bufs=2))
        qkTp = es_l.enter_context(tc.tile_pool(name="qkTp", bufs=1))
        vp = es_l.enter_context(tc.tile_pool(name="vp", bufs=1))
        ep = es_l.enter_context(tc.tile_pool(name="ep", bufs=3))
        yp = es_l.enter_context(tc.tile_pool(name="yp", bufs=1))
        sm = es_l.enter_context(tc.tile_pool(name="sm", bufs=1))
        fcwp = es_l.enter_context(tc.tile_pool(name="fcwp", bufs=4))
        mTp = es_l.enter_context(tc.tile_pool(name="mTp", bufs=1))
        prwp = es_l.enter_context(tc.tile_pool(name="prwp", bufs=3))

        for layer in range(L):
            ln1g_bc = bcast_row(lnrow, ln1g[layer], 128, F32R, E)
            ln1b_bc = bcast_row(lnrow, ln1b[layer], 128, F32R, E)

            # ---- LN1 + transpose -> hT_sb [768, 256] ----
            hT_sb = [hTp.tile([128, TS], F32R, tag=f"hT{k}", name=f"hT{k}") for k in range(6)]
            for t in range(2):
                h_t = hpool.tile([128, E], F32R, tag="h", name="h")
                layernorm_t(x_sb[t][:], ln1g_bc, ln1b_bc, h_t)
                transpose_to(h_t, hT_sb, t * 128)
            for k in range(6):
                nc.sync.dma_start(out=hT_in[k * 128:(k + 1) * 128, :].bitcast(F32R),
                                  in_=hT_sb[k][:])

            # ---- AllGather hidden (within batch group of 4) ----
            nc.gpsimd.collective_compute(
                "AllGather", mybir.AluOpType.bypass,
                replica_groups=g_batch,
                ins=[hT_in.opt()],
                outs=[hT_ag[0:4 * E, :].opt()],
            )
            # load hT for my batch: 6 tiles [128, 1024]
            hTb = [hTbp.tile([128, T], F32R, tag=f"hTb{k}", name=f"hTb{k}") for k in range(6)]
            for k in range(6):
                for r in range(4):
                    nc.sync.dma_start(
                        out=hTb[k][:, r * TS:(r + 1) * TS],
                        in_=hT_ag[r * E + k * 128: r * E + (k + 1) * 128, :].bitcast(F32R))

            # ---- QKV ----
            es_a = ExitStack()
            psQ = es_a.enter_context(tc.tile_pool(name="psQ", bufs=2, space="PSUM"))
            psV = es_a.enter_context(tc.tile_pool(name="psV", bufs=1, space="PSUM"))
            psS = es_a.enter_context(tc.tile_pool(name="psS", bufs=2, space="PSUM"))
            psY = es_a.enter_context(tc.tile_pool(name="psY", bufs=1, space="PSUM"))
            wqk_sb = [wqkp.tile([128, 384], F32R, tag=f"wqk{k}", name=f"wqk{k}") for k in range(6)]
            for k in range(6):
                nc.sync.dma_start(out=wqk_sb[k][:],
                                  in_=wqk[layer, k * 128:(k + 1) * 128, :].bitcast(F32R))
            bqk_sb2 = []
            for m in range(6):
                bt = bias_p.tile([64, 1], F32, tag=f"bqk{m}", name=f"bqk{m}")
                nc.sync.dma_start(out=bt[:], in_=bqk[layer, m].unsqueeze(1))
                bqk_sb2.append(bt)
            qkT = [qkTp.tile([64, T], F32R, tag=f"qkT{m}", name=f"qkT{m}") for m in range(6)]
            for m in range(6):
                for n in range(2):
                    ps = psQ.tile([64, 512], F32, tag="q", name="q")
                    for k in range(6):
                        nc.tensor.matmul(ps[:], wqk_sb[k][:, m * 64:(m + 1) * 64],
                                         hTb[k][:, n * 512:(n + 1) * 512],
                                         start=(k == 0), stop=(k == 5))
                    nc.vector.tensor_scalar_add(out=qkT[m][:, n * 512:(n + 1) * 512],
                                                in0=ps[:],
                                                scalar1=bqk_sb2[m][:, 0:1])

            wv_sb = [wvp.tile([128, 256], F32R, tag=f"wv{k}", name=f"wv{k}") for k in range(6)]
            for k in range(6):
                nc.sync.dma_start(out=wv_sb[k][:],
                                  in_=wv[layer, k * 128:(k + 1) * 128, :].bitcast(F32R))
            v_sb = [vp.tile([128, 3 * 65], F32R, tag=f"v{t}", name=f"v{t}") for t in range(8)]
            for t in range(8):
                ps = psV.tile([128, 256], F32, tag="v", name="v")
                for k in range(6):
                    nc.tensor.matmul(ps[:], hTb[k][:, t * 128:(t + 1) * 128],
                                     wv_sb[k][:], start=(k == 0), stop=(k == 5))
                for h in range(3):
                    nc.scalar.activation(out=v_sb[t][:, 65 * h:65 * h + 64],
                                         in_=ps[:, 64 * h:64 * h + 64],
                                         func=mybir.ActivationFunctionType.Copy)
                    nc.vector.tensor_copy(v_sb[t][:, 65 * h + 64:65 * h + 65],
                                          ones_r[:])

            bv_sb = bias_p.tile([64, 3], F32, tag="bv", name="bv")
            nc.sync.dma_start(out=bv_sb[:], in_=bv[layer].transpose([1, 0]))

            # ---- attention per head ----
            yT_sb = []
            for h in range(3):
                qT = qkT[h][:, :]
                kT = qkT[3 + h][:, :]
                yps = psY.tile([65, T], F32, tag="y", name="y")
                for j in range(8):
                    qs = j * 128
                    qlen = T - qs
                    e_sb = ep.tile([128, T], F32R, tag="e", name="e")
                    off = 0
                    while off < qlen:
                        cl = min(512, qlen - off)
                        pss = psS.tile([128, 512], F32, tag="s", name="s")
                        nc.tensor.matmul(pss[:, 0:cl], kT[:, j * 128:(j + 1) * 128],
                                         qT[:, qs + off: qs + off + cl],
                                         start=True, stop=True)
                        nc.scalar.activation(out=e_sb[:, off:off + cl],
                                             in_=pss[:, 0:cl],
                                             func=mybir.ActivationFunctionType.Exp,
                                             scale=SCALE)
                        off += cl
                    nc.vector.tensor_mul(out=e_sb[:, 0:128], in0=e_sb[:, 0:128],
                                         in1=tri_sb[:])
                    # accumulate yT over k-blocks, per psum bank
                    if qs < 512:
                        nc.tensor.matmul(yps[:, qs:512],
                                         v_sb[j][:, 65 * h:65 * h + 65],
                                         e_sb[:, 0:512 - qs],
                                         start=(j == 0), stop=(j == 3))
                    nc.tensor.matmul(yps[:, max(qs, 512):T],
                                     v_sb[j][:, 65 * h:65 * h + 65],
                                     e_sb[:, max(qs, 512) - qs:qlen],
                                     start=(j == 0), stop=(j == 7))
                recip = sm.tile([1, T], F32, tag="recip", name="recip")
                nc.vector.reciprocal(out=recip[:], in_=yps[64:65, :])
                recip_bc = sm.tile([64, T], F32, tag="recip_bc", name="recip_bc")
                nc.gpsimd.partition_broadcast(recip_bc[:], recip[:])
                yT = yp.tile([64, T], F32R, tag=f"yT{h}", name=f"yT{h}")
                nc.vector.tensor_mul(out=yT[:], in0=yps[0:64, :], in1=recip_bc[:])
                nc.vector.tensor_scalar_add(out=yT[:], in0=yT[:],
                                            scalar1=bv_sb[:, h:h + 1])
                yT_sb.append(yT)

            # ---- out-proj partials -> ReduceScatter ----
            es_a.close()
            es_b = ExitStack()
            psO = es_b.enter_context(tc.tile_pool(name="psO", bufs=2, space="PSUM"))
            watp_sb = [watpp.tile([64, E], F32R, tag=f"watp{h}", name=f"watp{h}") for h in range(3)]
            for h in range(3):
                nc.sync.dma_start(out=watp_sb[h][:],
                                  in_=watp[layer, 64 * h:64 * h + 64, :].bitcast(F32R))
            for t in range(8):
                ps = psO.tile([128, E], F32, tag="o", name="o")
                for n0, n1 in ((0, 512), (512, 768)):
                    for h in range(3):
                        nc.tensor.matmul(ps[:, n0:n1],
                                         yT_sb[h][:, t * 128:(t + 1) * 128],
                                         watp_sb[h][:, n0:n1],
                                         start=(h == 0), stop=(h == 2))
                ao = hpool.tile([128, E], mybir.dt.bfloat16, tag="ao", name="ao")
                if t % 2 == 0:
                    nc.vector.tensor_copy(ao[:], ps[:])
                else:
                    nc.scalar.activation(out=ao[:], in_=ps[:],
                                         func=mybir.ActivationFunctionType.Copy)
                nc.sync.dma_start(out=rs_in[t * 128:(t + 1) * 128, :], in_=ao[:])
            nc.gpsimd.collective_compute(
                "ReduceScatter", mybir.AluOpType.add,
                replica_groups=g_batch,
                ins=[rs_in.opt()],
                outs=[rs_out.opt()],
            )
            atpb_bc = bcast_row(lnrow, atpb[layer], 128, F32, E)
            for t in range(2):
                rsb = hpool.tile([128, E], mybir.dt.bfloat16, tag="rsb", name="rsb")
                nc.sync.dma_start(out=rsb[:], in_=rs_out[t * 128:(t + 1) * 128, :])
                nc.vector.tensor_add(out=x_sb[t][:], in0=x_sb[t][:], in1=rsb[:])
                nc.vector.tensor_add(out=x_sb[t][:], in0=x_sb[t][:], in1=atpb_bc[:])

            # ---- LN2 + transpose ----
            ln2g_bc = bcast_row(lnrow, ln2g[layer], 128, F32R, E)
            ln2b_bc = bcast_row(lnrow, ln2b[layer], 128, F32R, E)
            h2T = [hTp.tile([128, TS], F32R, tag=f"hT{k}", name=f"hT{k}") for k in range(6)]
            for t in range(2):
                h_t = hpool.tile([128, E], F32R, tag="h", name="h")
                layernorm_t(x_sb[t][:], ln2g_bc, ln2b_bc, h_t)
                transpose_to(h_t, h2T, t * 128)

            # ---- MLP fc (mT = gelu(fc_w.T @ h2T + fc_b)) ----

            es_b.close()
            es_c = ExitStack()
            psM = es_c.enter_context(tc.tile_pool(name="psM", bufs=2, space="PSUM"))
            psP = es_c.enter_context(tc.tile_pool(name="psP", bufs=1, space="PSUM"))
            fcb_sb = bias_p.tile([128, 24], F32, tag="fcb", name="fcb")
            nc.sync.dma_start(out=fcb_sb[:], in_=fcb[layer].transpose([1, 0]))
            mT = [mTp.tile([128, TS], F32R, tag=f"mT{m}", name=f"mT{m}") for m in range(24)]
            for m in range(24):
                ps = psM.tile([128, TS], F32, tag="m", name="m")
                for k in range(6):
                    fck = fcwp.tile([128, 128], F32R, tag=f"fck{k}", name=f"fck{k}")
                    nc.sync.dma_start(
                        out=fck[:],
                        in_=fcw[layer, m, k].bitcast(F32R))
                    nc.tensor.matmul(ps[:], fck[:],
                                     h2T[k][:], start=(k == 0), stop=(k == 5))
                nc.scalar.activation(out=mT[m][:], in_=ps[:],
                                     func=mybir.ActivationFunctionType.Gelu_apprx_tanh,
                                     bias=fcb_sb[:, m:m + 1])

            # ---- MLP pr + residual ----
            prb_bc = bcast_row(lnrow, prb[layer], 128, F32, E)
            ps2 = [psP.tile([128, E], F32, tag=f"p{t}", name=f"p{t}") for t in range(2)]
            for k in range(24):
                prw_sb = prwp.tile([128, E], F32R, tag="prw", name="prw")
                nc.sync.dma_start(out=prw_sb[:],
                                  in_=prw[layer, k * 128:(k + 1) * 128, :].bitcast(F32R))
                for t in range(2):
                    for n0, n1 in ((0, 512), (512, 768)):
                        nc.tensor.matmul(ps2[t][:, n0:n1],
                                         mT[k][:, t * 128:(t + 1) * 128],
                                         prw_sb[:, n0:n1],
                                         start=(k == 0), stop=(k == 23))
            for t in range(2):
                nc.vector.tensor_add(out=x_sb[t][:], in0=x_sb[t][:], in1=ps2[t][:])
                nc.vector.tensor_add(out=x_sb[t][:], in0=x_sb[t][:], in1=prb_bc[:])
            es_c.close()

        # ---- final LN + AllGather(all 8) + lm_head ----
        lnfg_bc = bcast_row(lnrow, lnfg[0], 128, F32R, E)
        lnfb_bc = bcast_row(lnrow, lnfb[0], 128, F32R, E)
        xfT = [hTp.tile([128, TS], F32R, tag=f"hT{k}", name=f"hT{k}") for k in range(6)]
        for t in range(2):
            h_t = hpool.tile([128, E], F32R, tag="h", name="h")
            layernorm_t(x_sb[t][:], lnfg_bc, lnfb_bc, h_t)
            transpose_to(h_t, xfT, t * 128)
        for k in range(6):
            nc.sync.dma_start(out=xf_in[k * 128:(k + 1) * 128, :].bitcast(F32R),
                              in_=xfT[k][:])
        nc.gpsimd.collective_compute(
            "AllGather", mybir.AluOpType.bypass,
            replica_groups=g_all,
            ins=[xf_in.opt()],
            outs=[xf_ag.opt()],
        )
        es_l.close()
        es_h = es.enter_context(ExitStack())
        xfp = es_h.enter_context(tc.tile_pool(name="xfp", bufs=1))
        wtep = es_h.enter_context(tc.tile_pool(name="wtep", bufs=3))
        psL = es_h.enter_context(tc.tile_pool(name="psL", bufs=4, space="PSUM"))

        xf_sb = [xfp.tile([128, TS], F32R, tag=f"xf{i}", name=f"xf{i}") for i in range(48)]
        for i in range(48):
            nc.sync.dma_start(out=xf_sb[i][:],
                              in_=xf_ag[i * 128:(i + 1) * 128, :].bitcast(F32R))

        nch = (VS + 511) // 512
        for n in range(nch):
            n0 = n * 512
            nw = min(512, VS - n0)
            wte_sb = [wtep.tile([128, 512], F32R, tag=f"wte{k}", name=f"wte{k}") for k in range(6)]
            for k in range(6):
                nc.sync.dma_start(out=wte_sb[k][:, 0:nw],
                                  in_=wteT[k * 128:(k + 1) * 128, n0:n0 + nw].bitcast(F32R))
            for t in range(16):
                r, half = t // 2, t % 2
                ps = psL.tile([128, 512], F32, tag="l", name="l")
                for k in range(6):
                    nc.tensor.matmul(ps[:, 0:nw],
                                     xf_sb[r * 6 + k][:, half * 128:(half + 1) * 128],
                                     wte_sb[k][:, 0:nw],
                                     start=(k == 0), stop=(k == 5))
                lo = wtep.tile([128, 512], F32, tag="lo", name="lo")
                if t % 2 == 0:
                    nc.vector.tensor_copy(lo[:, 0:nw], ps[:, 0:nw])
                else:
                    nc.scalar.activation(out=lo[:, 0:nw], in_=ps[:, 0:nw],
                                         func=mybir.ActivationFunctionType.Copy)
                nc.sync.dma_start(out=logits[t * 128:(t + 1) * 128, n0:n0 + nw],
                                  in_=lo[:, 0:nw])

    nc.compile()
    return nc


def _prep_inputs(idx, wte, wpe, ln1_w, ln1_b, attn_w, attn_b, atp_w, atp_b,
                 ln2_w, ln2_b, fc_w, fc_b, pr_w, pr_b, lnf_w, lnf_b):
    idx = np.asarray(idx)
    f = lambda a: np.ascontiguousarray(np.asarray(a), dtype=np.float32)
    wte, wpe = f(wte), f(wpe)
    x0 = wte[idx.reshape(-1)] + np.tile(wpe[:T], (B, 1))  # [2048, 768]
    wte_pad = np.zeros((VPAD, E), np.float32)
    wte_pad[:V] = wte
    wteT_full = np.ascontiguousarray(wte_pad.T)  # [768, VPAD]

    attn_w, attn_b = f(attn_w), f(attn_b)
    atp_w, atp_b = f(atp_w), f(atp_b)
    fc_w, fc_b, pr_w, pr_b = f(fc_w), f(fc_b), f(pr_w), f(pr_b)
    # [L, 768, 3072] -> [L, 24(m), 6(k), 128, 128] contiguous tiles
    fcw_tiled = np.ascontiguousarray(
        fc_w.reshape(L, 6, 128, 24, 128).transpose(0, 3, 1, 2, 4))
    tri = (np.arange(128)[None, :] >= np.arange(128)[:, None]).astype(np.float32)

    in_maps = []
    for c in range(NC):
        hs = 3 * (c % 4)
        qcols = [attn_w[:, :, h * HD:(h + 1) * HD] for h in range(hs, hs + 3)]
        kcols = [attn_w[:, :, E + h * HD:E + (h + 1) * HD] for h in range(hs, hs + 3)]
        vcols = [attn_w[:, :, 2 * E + h * HD:2 * E + (h + 1) * HD] for h in range(hs, hs + 3)]
        wqk_c = np.ascontiguousarray(np.concatenate(qcols + kcols, axis=2))
        wv_c = np.concatenate(vcols, axis=2)
        wv_c = np.ascontiguousarray(
            np.concatenate([wv_c, np.zeros((L, E, 64), np.float32)], axis=2))
        bq = [attn_b[:, h * HD:(h + 1) * HD] for h in range(hs, hs + 3)]
        bk = [attn_b[:, E + h * HD:E + (h + 1) * HD] for h in range(hs, hs + 3)]
        bvs = [attn_b[:, 2 * E + h * HD:2 * E + (h + 1) * HD] for h in range(hs, hs + 3)]
        bqk_c = np.concatenate(bq + bk, axis=1).reshape(L, 6, 64)
        bv_c = np.stack(bvs, axis=1)  # [L, 3, 64]
        watp_c = np.ascontiguousarray(atp_w[:, hs * HD:(hs + 3) * HD, :])
        in_maps.append({
            "x0s": np.ascontiguousarray(x0[c * TS:(c + 1) * TS]),
            "wqk": wqk_c, "bqk": np.ascontiguousarray(bqk_c),
            "wv": wv_c, "bv": np.ascontiguousarray(bv_c),
            "watp": watp_c, "atpb": atp_b,
            "fcw": fcw_tiled, "fcb": np.ascontiguousarray(fc_b.reshape(L, 24, 128)),
            "prw": pr_w, "prb": pr_b,
            "ln1g": f(ln1_w), "ln1b": f(ln1_b),
            "ln2g": f(ln2_w), "ln2b": f(ln2_b),
            "lnfg": f(lnf_w).reshape(1, E), "lnfb": f(lnf_b).reshape(1, E),
            "tri": tri,
            "wteT": np.ascontiguousarray(wteT_full[:, c * VS:(c + 1) * VS]),
        })
    return in_maps


def kernel(trace=False, **inputs):
    if "nc" not in _CACHE:
        _CACHE["nc"] = _build_program()
    nc = _CACHE["nc"]
    in_maps = _prep_inputs(**inputs)
    res = run_bass_kernel_spmd(nc, in_maps, core_ids=list(range(NC)), trace=trace)
    _CACHE["last_result"] = res
    logits = np.concatenate([res.results[c]["logits"] for c in range(NC)], axis=1)
    return logits[:, :V].reshape(B, T, V).astype(np.float32)



# revision 24
# speedup vs baseline: 1.5788x; 1.5788x over previous
"""GPT (4-layer, E=768, H=12, T=1024, B=2, V=50257) forward on 8 trn2 cores.

Sharding:
  - Residual stream x token-sharded fp32: core c owns tokens [c*256,(c+1)*256)
    of the flattened [2048] (batch-major): cores 0-3 = batch 0, 4-7 = batch 1.
  - Attention head-sharded within each batch group of 4 cores (3 heads each):
    AllGather hidden (fp16, split in 2 halves for overlap), compute q/k/v +
    scores + y for my heads over all 1024 tokens, then AllToAll the normalized
    per-head y back to token owners (uniform SPMD), out-proj token-local with
    full 768 contraction (fp32 psum, no low-precision reduction anywhere).
  - MLP fully token-local: fc weight-stationary (out [hid,tok]),
    pr activation-stationary (out [tok,E]) - no transposes inside MLP.
  - lm_head vocab-sharded fp16: AllGather lnf(x) (all 8), each core computes
    [2048, 6284] logit slice, vocab-group-outer loop with resident xfT.
  - All matmul inputs fp16 (fp32 psum accumulate); scores 2-head row-packed
    (tile_position concurrency); h2 on row-group 64-127.
"""

import sys
from contextlib import ExitStack
import numpy as np

sys.path.insert(0, "/opt/trn_rl_repo")

import concourse.bass as bass
import concourse.mybir as mybir
import concourse.tile as tile
from concourse import bacc
from concourse.bass_utils import run_bass_kernel_spmd
from concourse.masks import make_identity

L, H, E, T, V = 4, 12, 768, 1024, 50257
B = 2
NC = 8
TS = (B * T) // NC          # 256 tokens per core
VS = 6284                   # vocab slice per core (padded V = 50272)
VPAD = VS * NC
HD = 64
EPS = 1e-5
SCALE = float(1.0 / np.sqrt(np.float32(E)))
F32 = mybir.dt.float32
F16 = mybir.dt.float16

L_RUN = L  # layers actually executed (tests may truncate)
SIM_GELU = False  # sim lacks Gelu_apprx_tanh; use x*sigmoid(1.702x) for debug
DEBUG_DUMP = False  # dump layer-0 intermediates to a "dbg" output
_CACHE = {}


def _build_program():
    nc = bacc.Bacc("TRN2", target_bir_lowering=False, debug=False, num_devices=NC)

    # ---- I/O -------------------------------------------------------------
    x0s = nc.dram_tensor("x0s", [TS, E], F32, kind="ExternalInput")
    # wqk cols: [q0|q1 (128), k0|k1 (128), pad|q2 (128), pad|k2 (128)]
    wqk = nc.dram_tensor("wqk", [L, E, 512], F16, kind="ExternalInput")
    bqk = nc.dram_tensor("bqk", [L, 128, 4], F32, kind="ExternalInput")
    wv = nc.dram_tensor("wv", [L, E, 3 * HD], F16, kind="ExternalInput")
    watp = nc.dram_tensor("watp", [L, 3 * HD, E], F16, kind="ExternalInput")  # my heads' rows
    atpb = nc.dram_tensor("atpb", [L, E], F32, kind="ExternalInput")     # includes bv@watp fold
    fcw = nc.dram_tensor("fcw", [L, E, 4 * E], F16, kind="ExternalInput")
    fcb = nc.dram_tensor("fcb", [L, 128, 24], F32, kind="ExternalInput")
    prw = nc.dram_tensor("prw", [L, 4 * E, E], F16, kind="ExternalInput")
    prb = nc.dram_tensor("prb", [L, E], F32, kind="ExternalInput")
    ln1g = nc.dram_tensor("ln1g", [L, E], F32, kind="ExternalInput")
    ln1b = nc.dram_tensor("ln1b", [L, E], F32, kind="ExternalInput")
    ln2g = nc.dram_tensor("ln2g", [L, E], F32, kind="ExternalInput")
    ln2b = nc.dram_tensor("ln2b", [L, E], F32, kind="ExternalInput")
    lnfg = nc.dram_tensor("lnfg", [1, E], F32, kind="ExternalInput")
    lnfb = nc.dram_tensor("lnfb", [1, E], F32, kind="ExternalInput")
    wteT = nc.dram_tensor("wteT", [E, VS], F16, kind="ExternalInput")
    logits = nc.dram_tensor("logits", [B * T, VS], F16, kind="ExternalOutput")
    dbg = (nc.dram_tensor("dbg", [12, 128, T], F16, kind="ExternalOutput")
           if DEBUG_DUMP else None)

    tri_np = (np.arange(128)[None, :] >= np.arange(128)[:, None]).astype(np.float16)
    tri = nc.inline_tensor(tri_np, name="tri_const")

    g_all = [list(range(NC))]
    g_batch = [[0, 1, 2, 3], [4, 5, 6, 7]]

    def bcast_row(pool, src_ap, n, dtype, w, tag=None):
        """Replicate a [w] DRAM row across n partitions via broadcast DMA."""
        t = pool.tile([n, w], dtype, tag=tag)
        in_ap = bass.AP(
            tensor=src_ap.tensor,
            offset=src_ap.offset,
            ap=[[0, n]] + [list(p) for p in src_ap.ap],
        )
        eng = nc.gpsimd if dtype != src_ap.dtype else nc.sync
        eng.dma_start(out=t[:], in_=in_ap)
        return t

    with tile.TileContext(nc) as tc, ExitStack() as es:
        const = es.enter_context(tc.tile_pool(name="const", bufs=1))
        xp = es.enter_context(tc.tile_pool(name="xp", bufs=1))
        lnrow = es.enter_context(tc.tile_pool(name="lnrow", bufs=1))
        stat = es.enter_context(tc.tile_pool(name="stat", bufs=2))
        hpool = es.enter_context(tc.tile_pool(name="hpool", bufs=2))
        dram = es.enter_context(tc.tile_pool(name="dram", bufs=1, space="DRAM"))

        ident_f = const.tile([128, 128], F32, name="ident_f")
        make_identity(nc, ident_f)
        ident = const.tile([128, 128], F16, name="ident")
        nc.vector.tensor_copy(ident[:], ident_f[:])
        tri_sb = const.tile([128, 128], F16, name="tri_sb")
        nc.sync.dma_start(out=tri_sb[:], in_=tri[:, :])
        eps_sb = const.tile([128, 1], F32, name="eps_sb")
        nc.vector.memset(eps_sb, EPS)
        ones3 = const.tile([128, 3], F16, name="ones3")
        nc.vector.memset(ones3, 1.0)

        # persistent residual stream [256, 768] fp32 as two [128, 768] tiles
        x_sb = [xp.tile([128, E], F32, tag=f"x{t}", name=f"x{t}") for t in range(2)]
        for t in range(2):
            nc.sync.dma_start(out=x_sb[t][:], in_=x0s[t * 128:(t + 1) * 128, :])

        # DRAM bounce buffers for collectives (fp16)
        hT_in = [dram.tile([3 * 128, TS], F16, name=f"hT_in{i}") for i in range(2)]
        hT_ag = [dram.tile([4 * 3 * 128, TS], F16, name=f"hT_ag{i}") for i in range(2)]
        rs_in = dram.tile([T, E], F16, name="rs_in")
        rs_out = dram.tile([TS, E], F16, name="rs_out")
        xf_in = dram.tile([E, TS], F16, name="xf_in")
        xf_ag = dram.tile([NC * E, TS], F16, name="xf_ag")

        def layernorm_t(x_ap, g_bc, b_bc, out_tile):
            """LN over free dim (768) of [128, 768] fp32 tile -> out fp16."""
            stats = stat.tile([128, 3, 6], F32, tag="bn_stats", name="bn_stats_t")
            xr = x_ap.rearrange("p (s d) -> p s d", s=3)
            for s in range(3):
                nc.vector.bn_stats(out=stats[:, s, :], in_=xr[:, s, :])
            mv = stat.tile([128, 2], F32, tag="bn_aggr", name="bn_aggr_t")
            nc.vector.bn_aggr(out=mv[:], in_=stats[:])
            rstd = stat.tile([128, 1], F32, tag="rstd", name="rstd_t")
            nc.scalar.activation(out=rstd[:], in_=mv[:, 1:2],
                                 func=mybir.ActivationFunctionType.Sqrt,
                                 bias=eps_sb[:], scale=1.0)
            nc.vector.reciprocal(out=rstd[:], in_=rstd[:])
            tmp = stat.tile([128, E], F32, tag="ln_tmp", name="ln_tmp")
            nc.vector.tensor_scalar(out=tmp[:], in0=x_ap,
                                    scalar1=mv[:, 0:1], scalar2=rstd[:],
                                    op0=mybir.AluOpType.subtract,
                                    op1=mybir.AluOpType.mult)
            nc.vector.tensor_mul(out=tmp[:], in0=tmp[:], in1=g_bc[:])
            nc.vector.tensor_add(out=out_tile[:], in0=tmp[:], in1=b_bc[:])

        # ---- persistent layer pools (tags reused across layers) ----------
        es_l = es.enter_context(ExitStack())
        wqkp = es_l.enter_context(tc.tile_pool(name="wqkp", bufs=1))
        wvp = es_l.enter_context(tc.tile_pool(name="wvp", bufs=1))
        watpp = es_l.enter_context(tc.tile_pool(name="watpp", bufs=1))
        fcwp = es_l.enter_context(tc.tile_pool(name="fcwp", bufs=1))
        prwp = es_l.enter_context(tc.tile_pool(name="prwp", bufs=1))
        bias_p = es_l.enter_context(tc.tile_pool(name="bias_p", bufs=2))
        hTp = es_l.enter_context(tc.tile_pool(name="hTp", bufs=1))
        hTbp = es_l.enter_context(tc.tile_pool(name="hTbp", bufs=1))
        qkp = es_l.enter_context(tc.tile_pool(name="qkp", bufs=1))
        vp = es_l.enter_context(tc.tile_pool(name="vp", bufs=1))
        ep = es_l.enter_context(tc.tile_pool(name="ep", bufs=1))
        yp = es_l.enter_context(tc.tile_pool(name="yp", bufs=1))
        sm = es_l.enter_context(tc.tile_pool(name="sm", bufs=1))
        mTp = es_l.enter_context(tc.tile_pool(name="mTp", bufs=1))

        for layer in range(L_RUN):
            # ---- weight loads (gpsimd queue; Tile schedules early) ------
            wqk_sb = [wqkp.tile([128, 512], F16, tag=f"wqk{k}", name=f"wqk{k}") for k in range(6)]
            wv_sb = [wvp.tile([128, 192], F16, tag=f"wv{k}", name=f"wv{k}") for k in range(6)]
            wa0 = watpp.tile([128, E], F16, tag="wa0", name="wa0")
            wa1 = watpp.tile([64, E], F16, tag="wa1", name="wa1")
            nc.gpsimd.dma_start(out=wa0[:], in_=watp[layer, 0:128, :])
            nc.gpsimd.dma_start(out=wa1[:], in_=watp[layer, 128:192, :])
            for k in range(6):
                nc.gpsimd.dma_start(out=wqk_sb[k][:], in_=wqk[layer, k * 128:(k + 1) * 128, :])
                nc.gpsimd.dma_start(out=wv_sb[k][:], in_=wv[layer, k * 128:(k + 1) * 128, :])
            fcw_sb = [fcwp.tile([128, 4 * E], F16, tag=f"fcw{k}", name=f"fcw{k}") for k in range(6)]
            for k in range(6):
                nc.gpsimd.dma_start(out=fcw_sb[k][:], in_=fcw[layer, k * 128:(k + 1) * 128, :])
            prw_sb = [prwp.tile([128, E], F16, tag=f"prw{m}", name=f"prw{m}") for m in range(24)]
            for m in range(24):
                nc.gpsimd.dma_start(out=prw_sb[m][:], in_=prw[layer, m * 128:(m + 1) * 128, :])
            bqk_sb = bias_p.tile([128, 4], F32, tag="bqk", name="bqk")
            nc.sync.dma_start(out=bqk_sb[:], in_=bqk[layer])
            fcb_sb = bias_p.tile([128, 24], F32, tag="fcb", name="fcb")
            nc.sync.dma_start(out=fcb_sb[:], in_=fcb[layer])
            ln1g_bc = bcast_row(lnrow, ln1g[layer], 128, F16, E, tag="ln1g")
            ln1b_bc = bcast_row(lnrow, ln1b[layer], 128, F16, E, tag="ln1b")

            # ---- LN1 + transpose -> hT [768, 256] fp16 -------------------
            es_t1 = ExitStack()
            psT = es_t1.enter_context(tc.tile_pool(name="psT", bufs=4, space="PSUM"))
            hT = [hTp.tile([128, TS], F16, tag=f"hT{k}", name=f"hT{k}") for k in range(6)]
            for t in range(2):
                h_t = hpool.tile([128, E], F16, tag="h", name="h")
                layernorm_t(x_sb[t][:], ln1g_bc, ln1b_bc, h_t)
                for k in range(6):
                    pt = psT.tile([128, 128], F16, tag="tr", name="tr")
                    nc.tensor.transpose(pt[:], h_t[:, k * 128:(k + 1) * 128], ident[:])
                    dst = hT[k][:, t * 128:(t + 1) * 128]
                    if k % 2 == 0:
                        nc.vector.tensor_copy(dst, pt[:])
                    else:
                        nc.scalar.activation(out=dst, in_=pt[:],
                                             func=mybir.ActivationFunctionType.Copy)
            es_t1.close()

            # ---- AllGather hidden in 2 halves (within batch group of 4) --
            for half in range(2):
                for k in range(3):
                    nc.sync.dma_start(out=hT_in[half][k * 128:(k + 1) * 128, :],
                                      in_=hT[half * 3 + k][:])
                nc.gpsimd.collective_compute(
                    "AllGather", mybir.AluOpType.bypass,
                    replica_groups=g_batch,
                    ins=[hT_in[half].opt()],
                    outs=[hT_ag[half].opt()],
                )
            # load hTb 6 x [128, 1024] fp16 (one 3D-AP DMA per k-chunk)
            hTb = [hTbp.tile([128, T], F16, tag=f"hTb{k}", name=f"hTb{k}") for k in range(6)]
            for k in range(6):
                half, kk = divmod(k, 3)
                src = hT_ag[half]
                in_ap = bass.AP(
                    tensor=src.tensor,
                    offset=src[kk * 128, 0].offset,
                    ap=[[TS, 128], [3 * 128 * TS, 4], [1, TS]],
                )
                nc.sync.dma_start(
                    out=hTb[k][:].rearrange("p (r t) -> p r t", r=4),
                    in_=in_ap)

            if DEBUG_DUMP and layer == 0:
                nc.sync.dma_start(out=dbg[0], in_=hTb[0][:])
                nc.sync.dma_start(out=dbg[11][:, 0:128], in_=tri_sb[:])

            # ---- QKV ----------------------------------------------------
            es_a = ExitStack()
            psQK = es_a.enter_context(tc.tile_pool(name="psQK", bufs=3, space="PSUM"))
            psV = es_a.enter_context(tc.tile_pool(name="psV", bufs=2, space="PSUM"))
            # qT2/kT2: rows 0-63 head0, 64-127 head1; q1T/k1T rows 64-127 head2
            qT2 = qkp.tile([128, T], F16, tag="qT2", name="qT2")
            kT2 = qkp.tile([128, T], F16, tag="kT2", name="kT2")
            q1T = qkp.tile([128, T], F16, tag="q1T", name="q1T")
            k1T = qkp.tile([128, T], F16, tag="k1T", name="k1T")
            qk_dsts = [qT2, kT2, q1T, k1T]
            for s in range(4):
                ps = psQK.tile([128, T], F32, tag="qk", name="qk")
                for n in range(2):
                    for k in range(6):
                        nc.tensor.matmul(ps[:, n * 512:(n + 1) * 512],
                                         wqk_sb[k][:, s * 128:(s + 1) * 128],
                                         hTb[k][:, n * 512:(n + 1) * 512],
                                         start=(k == 0), stop=(k == 5))
                if s < 2:
                    nc.vector.tensor_scalar_add(out=qk_dsts[s][:], in0=ps[:],
                                                scalar1=bqk_sb[:, s:s + 1])
                else:
                    nc.vector.tensor_scalar_add(out=qk_dsts[s][64:128, :],
                                                in0=ps[64:128, :],
                                                scalar1=bqk_sb[64:128, s:s + 1])
            # v_sb layout [128, 258]: h0 [v0|1] at 0:65, h1 [1|pad63|v1] at
            # 65:193 (den at psum row 0, y1 at rows 64-127 for packed
            # out-proj), h2 [v2|1] at 193:258. Pad never read downstream.
            v_sb = [vp.tile([128, 258], F16, tag=f"v{t}", name=f"v{t}") for t in range(8)]
            for t in range(8):
                ps = psV.tile([128, 192], F32, tag="v", name="v")
                for k in range(6):
                    nc.tensor.matmul(ps[:], hTb[k][:, t * 128:(t + 1) * 128],
                                     wv_sb[k][:], start=(k == 0), stop=(k == 5))
                nc.vector.tensor_copy(v_sb[t][:, 0:64], ps[:, 0:64])
                nc.vector.tensor_copy(v_sb[t][:, 129:193], ps[:, 64:128])
                nc.vector.tensor_copy(v_sb[t][:, 193:257], ps[:, 128:192])
                nc.vector.tensor_copy(v_sb[t][:, 64:65], ones3[:, 0:1])
                nc.vector.tensor_copy(v_sb[t][:, 65:66], ones3[:, 1:2])
                nc.vector.tensor_copy(v_sb[t][:, 257:258], ones3[:, 2:3])
                nc.vector.memset(v_sb[t][:, 66:129], 0.0)

            if DEBUG_DUMP and layer == 0:
                nc.sync.dma_start(out=dbg[1], in_=qT2[:])
                nc.sync.dma_start(out=dbg[2], in_=kT2[:])
                nc.sync.dma_start(out=dbg[3], in_=k1T[:])
                nc.sync.dma_start(out=dbg[10][:, 0:258], in_=v_sb[0][:])

            # ---- attention: scores + y, head 0/1 row-packed --------------
            es_a.close()
            es_b = ExitStack()
            psS = es_b.enter_context(tc.tile_pool(name="psS", bufs=2, space="PSUM"))
            psY = es_b.enter_context(tc.tile_pool(name="psY", bufs=1, space="PSUM"))
            # h0: y rows 0:64, den 64; h1: den 0, y 64:128; h2: y 0:64, den 64
            yps0 = psY.tile([65, T], F32, tag="y0", name="y0")
            yps1 = psY.tile([128, T], F32, tag="y1", name="y1")
            yps2 = psY.tile([65, T], F32, tag="y2", name="y2")
            heads = [(qT2[0:64, :], kT2[0:64, :], yps0, 0, 65),
                     (qT2[64:128, :], kT2[64:128, :], yps1, 65, 193),
                     (q1T[64:128, :], k1T[64:128, :], yps2, 193, 258)]
            for j in range(8):
                qs = j * 128
                qlen = T - qs
                for h in range(3):
                    qT_h, kT_h, yout, v0, v1 = heads[h]
                    e_sb = ep.tile([128, T], F16, tag=f"e{h}", name=f"e{h}")
                    off = 0
                    while off < qlen:
                        cl = min(512, qlen - off)
                        pss = psS.tile([128, 512], F32, tag="s", name="s")
                        nc.tensor.matmul(pss[:, 0:cl], kT_h[:, qs:qs + 128],
                                         qT_h[:, qs + off: qs + off + cl],
                                         start=True, stop=True)
                        nc.scalar.activation(out=e_sb[:, off:off + cl],
                                             in_=pss[:, 0:cl],
                                             func=mybir.ActivationFunctionType.Exp,
                                             scale=SCALE)
                        off += cl
                    nc.vector.tensor_mul(out=e_sb[:, 0:128], in0=e_sb[:, 0:128],
                                         in1=tri_sb[:])
                    if qs < 512:
                        nc.tensor.matmul(yout[:, qs:512],
                                         v_sb[j][:, v0:v1],
                                         e_sb[:, 0:512 - qs],
                                         start=(j == 0), stop=(j == 3))
                    nc.tensor.matmul(yout[:, max(qs, 512):T],
                                     v_sb[j][:, v0:v1],
                                     e_sb[:, max(qs, 512) - qs:qlen],
                                     start=(j == 0), stop=(j == 7))

            # ---- normalize into packed y tiles ---------------------------
            # y01T rows 0:64 = head0, 64:128 = head1; y2T rows 0:64 = head2
            y01T = yp.tile([128, T], F16, tag="y01T", name="y01T")
            y2T = yp.tile([64, T], F16, tag="y2T", name="y2T")
            for h, (yout, den_row, dst) in enumerate(
                    [(yps0, 64, y01T[0:64, :]), (yps1, 0, y01T[64:128, :]),
                     (yps2, 64, y2T[:])]):
                recip = sm.tile([1, T], F32, tag="recip", name="recip")
                nc.vector.reciprocal(out=recip[:], in_=yout[den_row:den_row + 1, :])
                # broadcast to all 128 partitions (HW partition_broadcast
                # mishandles a base-64 target), then use the aligned slice
                bcf = sm.tile([128, T], F32, tag="bcf", name="bcf")
                nc.gpsimd.partition_broadcast(bcf[:], recip[:])
                lo = 64 if h == 1 else 0
                ysrc = yout[64:128, :] if h == 1 else yout[0:64, :]
                nc.vector.tensor_mul(out=dst, in0=ysrc, in1=bcf[lo:lo + 64, :])
            if DEBUG_DUMP and layer == 0:
                nc.sync.dma_start(out=dbg[4], in_=y01T[:])
                nc.sync.dma_start(out=dbg[5][0:64, :], in_=y2T[:])
            es_b.close()

            # ---- out-proj partials (all 1024 tokens) -> ReduceScatter ----
            es_c = ExitStack()
            psO = es_c.enter_context(tc.tile_pool(name="psO", bufs=3, space="PSUM"))
            atpb_bc = bcast_row(lnrow, atpb[layer], 128, F16, E, tag="atpb")
            for t in range(8):
                po = psO.tile([128, E], F32, tag="o", name="o")
                for n0, n1 in ((0, 512), (512, 768)):
                    nc.tensor.matmul(po[:, n0:n1],
                                     y01T[:, t * 128:(t + 1) * 128],
                                     wa0[:, n0:n1], start=True, stop=False)
                    nc.tensor.matmul(po[:, n0:n1],
                                     y2T[:, t * 128:(t + 1) * 128],
                                     wa1[:, n0:n1], start=False, stop=True)
                ao = hpool.tile([128, E], F16, tag="ao", name="ao")
                if t % 2 == 0:
                    nc.vector.tensor_copy(ao[:], po[:])
                else:
                    nc.scalar.activation(out=ao[:], in_=po[:],
                                         func=mybir.ActivationFunctionType.Copy)
                nc.sync.dma_start(out=rs_in[t * 128:(t + 1) * 128, :], in_=ao[:])
            nc.gpsimd.collective_compute(
                "ReduceScatter", mybir.AluOpType.add,
                replica_groups=g_batch,
                ins=[rs_in.opt()],
                outs=[rs_out.opt()],
            )
            for t in range(2):
                rsb = hpool.tile([128, E], F16, tag="rsb", name="rsb")
                nc.sync.dma_start(out=rsb[:], in_=rs_out[t * 128:(t + 1) * 128, :])
                if DEBUG_DUMP and layer == 0:
                    nc.sync.dma_start(out=dbg[6 + t][:, 0:E], in_=rsb[:])
                nc.vector.tensor_add(out=x_sb[t][:], in0=x_sb[t][:], in1=rsb[:])
                nc.vector.tensor_add(out=x_sb[t][:], in0=x_sb[t][:], in1=atpb_bc[:])
            if DEBUG_DUMP and layer == 0:
                for t in range(2):
                    xc = hpool.tile([128, E], F16, tag="ao", name="xc")
                    nc.vector.tensor_copy(xc[:], x_sb[t][:])
                    nc.sync.dma_start(out=dbg[8 + t][:, 0:E], in_=xc[:])
            es_c.close()

            # ---- LN2 + transpose -> h2T ---------------------------------
            ln2g_bc = bcast_row(lnrow, ln2g[layer], 128, F16, E, tag="ln2g")
            ln2b_bc = bcast_row(lnrow, ln2b[layer], 128, F16, E, tag="ln2b")
            es_t2 = ExitStack()
            psT2 = es_t2.enter_context(tc.tile_pool(name="psT2", bufs=4, space="PSUM"))
            h2T = [hTp.tile([128, TS], F16, tag=f"h2T{k}", name=f"h2T{k}") for k in range(6)]
            for t in range(2):
                h_t = hpool.tile([128, E], F16, tag="h", name="h")
                layernorm_t(x_sb[t][:], ln2g_bc, ln2b_bc, h_t)
                for k in range(6):
                    pt = psT2.tile([128, 128], F16, tag="tr2", name="tr2")
                    nc.tensor.transpose(pt[:], h_t[:, k * 128:(k + 1) * 128], ident[:])
                    dst = h2T[k][:, t * 128:(t + 1) * 128]
                    if k % 2 == 0:
                        nc.vector.tensor_copy(dst, pt[:])
                    else:
                        nc.scalar.activation(out=dst, in_=pt[:],
                                             func=mybir.ActivationFunctionType.Copy)
            es_t2.close()

            # ---- MLP fc: mT[m] = gelu(fcw[:,m].T @ h2T + fcb[m]) ---------
            es_d = ExitStack()
            psM = es_d.enter_context(tc.tile_pool(name="psM", bufs=4, space="PSUM"))
            psP = es_d.enter_context(tc.tile_pool(name="psP", bufs=1, space="PSUM"))
            mT = [mTp.tile([128, TS], F16, tag=f"mT{m}", name=f"mT{m}") for m in range(24)]
            if SIM_GELU:
                fcb17 = bias_p.tile([128, 24], F32, tag="fcb17", name="fcb17")
                nc.vector.tensor_scalar(out=fcb17[:], in0=fcb_sb[:], scalar1=1.702,
                                        scalar2=None,
                                        op0=mybir.AluOpType.mult)
            for m in range(24):
                ps = psM.tile([128, TS], F32, tag="m", name="m")
                for k in range(6):
                    nc.tensor.matmul(ps[:], fcw_sb[k][:, m * 128:(m + 1) * 128],
                                     h2T[k][:], start=(k == 0), stop=(k == 5))
                if SIM_GELU:
                    sgm = mTp.tile([128, TS], F32, tag="sgm", name="sgm")
                    nc.scalar.activation(out=sgm[:], in_=ps[:],
                                         func=mybir.ActivationFunctionType.Sigmoid,
                                         scale=1.702, bias=fcb17[:, m:m + 1])
                    nc.vector.tensor_scalar_add(out=mT[m][:], in0=ps[:],
                                                scalar1=fcb_sb[:, m:m + 1])
                    nc.vector.tensor_mul(out=mT[m][:], in0=mT[m][:], in1=sgm[:])
                else:
                    nc.scalar.activation(out=mT[m][:], in_=ps[:],
                                         func=mybir.ActivationFunctionType.Gelu_apprx_tanh,
                                         bias=fcb_sb[:, m:m + 1])

            # ---- MLP pr + residual --------------------------------------
            prb_bc = bcast_row(lnrow, prb[layer], 128, F16, E, tag="prb")
            ps2 = [psP.tile([128, E], F32, tag=f"p{t}", name=f"p{t}") for t in range(2)]
            for m in range(24):
                for t in range(2):
                    for n0, n1 in ((0, 512), (512, 768)):
                        nc.tensor.matmul(ps2[t][:, n0:n1],
                                         mT[m][:, t * 128:(t + 1) * 128],
                                         prw_sb[m][:, n0:n1],
                                         start=(m == 0), stop=(m == 23))
            for t in range(2):
                nc.vector.tensor_add(out=x_sb[t][:], in0=x_sb[t][:], in1=ps2[t][:])
                nc.vector.tensor_add(out=x_sb[t][:], in0=x_sb[t][:], in1=prb_bc[:])
            es_d.close()

        # ---- final LN + AllGather(all 8) + lm_head -----------------------
        lnfg_bc = bcast_row(lnrow, lnfg[0], 128, F16, E, tag="lnfg")
        lnfb_bc = bcast_row(lnrow, lnfb[0], 128, F16, E, tag="lnfb")
        es_tf = ExitStack()
        psTf = es_tf.enter_context(tc.tile_pool(name="psTf", bufs=4, space="PSUM"))
        xfT = [hTp.tile([128, TS], F16, tag=f"hT{k}", name=f"xfT{k}") for k in range(6)]
        for t in range(2):
            h_t = hpool.tile([128, E], F16, tag="h", name="h")
            layernorm_t(x_sb[t][:], lnfg_bc, lnfb_bc, h_t)
            for k in range(6):
                pt = psTf.tile([128, 128], F16, tag="trf", name="trf")
                nc.tensor.transpose(pt[:], h_t[:, k * 128:(k + 1) * 128], ident[:])
                dst = xfT[k][:, t * 128:(t + 1) * 128]
                if k % 2 == 0:
                    nc.vector.tensor_copy(dst, pt[:])
                else:
                    nc.scalar.activation(out=dst, in_=pt[:],
                                         func=mybir.ActivationFunctionType.Copy)
        es_tf.close()
        for k in range(6):
            nc.sync.dma_start(out=xf_in[k * 128:(k + 1) * 128, :], in_=xfT[k][:])
        nc.gpsimd.collective_compute(
            "AllGather", mybir.AluOpType.bypass,
            replica_groups=g_all,
            ins=[xf_in.opt()],
            outs=[xf_ag.opt()],
        )
        es_l.close()
        es_h = es.enter_context(ExitStack())
        xfp = es_h.enter_context(tc.tile_pool(name="xfp", bufs=1))
        wtep = es_h.enter_context(tc.tile_pool(name="wtep", bufs=2))
        lop = es_h.enter_context(tc.tile_pool(name="lop", bufs=4))
        psL = es_h.enter_context(tc.tile_pool(name="psL", bufs=6, space="PSUM"))

        # xfT_full 6 x [128, 2048] fp16 (3D-AP load across 8 ranks)
        xf_full = [xfp.tile([128, B * T], F16, tag=f"xf{k}", name=f"xf{k}") for k in range(6)]
        for k in range(6):
            in_ap = bass.AP(
                tensor=xf_ag.tensor,
                offset=xf_ag[k * 128, 0].offset,
                ap=[[TS, 128], [E * TS, NC], [1, TS]],
            )
            nc.sync.dma_start(out=xf_full[k][:].rearrange("p (r t) -> p r t", r=NC),
                              in_=in_ap)

        nch = (VS + 511) // 512
        for n in range(nch):
            n0 = n * 512
            nw = min(512, VS - n0)
            wte_sb = [wtep.tile([128, 512], F16, tag=f"wte{k}", name=f"wte{k}") for k in range(6)]
            for k in range(6):
                nc.gpsimd.dma_start(out=wte_sb[k][:, 0:nw],
                                    in_=wteT[k * 128:(k + 1) * 128, n0:n0 + nw])
            for t in range(16):
                ps = psL.tile([128, 512], F32, tag="l", name="l")
                for k in range(6):
                    nc.tensor.matmul(ps[:, 0:nw],
                                     xf_full[k][:, t * 128:(t + 1) * 128],
                                     wte_sb[k][:, 0:nw],
                                     start=(k == 0), stop=(k == 5))
                lo = lop.tile([128, 512], F16, tag="lo", name="lo")
                if t % 2 == 0:
                    nc.vector.tensor_copy(lo[:, 0:nw], ps[:, 0:nw])
                else:
                    nc.scalar.activation(out=lo[:, 0:nw], in_=ps[:, 0:nw],
                                         func=mybir.ActivationFunctionType.Copy)
                nc.sync.dma_start(out=logits[t * 128:(t + 1) * 128, n0:n0 + nw],
                                  in_=lo[:, 0:nw])

    nc.compile()
    return nc


def _prep_inputs(idx, wte, wpe, ln1_w, ln1_b, attn_w, attn_b, atp_w, atp_b,
                 ln2_w, ln2_b, fc_w, fc_b, pr_w, pr_b, lnf_w, lnf_b):
    idx = np.asarray(idx)
    f = lambda a: np.ascontiguousarray(np.asarray(a), dtype=np.float32)
    h = lambda a: np.ascontiguousarray(np.asarray(a), dtype=np.float16)
    wte, wpe = f(wte), f(wpe)
    x0 = wte[idx.reshape(-1)] + np.tile(wpe[:T], (B, 1))  # [2048, 768]
    wte_pad = np.zeros((VPAD, E), np.float32)
    wte_pad[:V] = wte
    wteT_full = np.ascontiguousarray(wte_pad.T)  # [768, VPAD]

    attn_w, attn_b = f(attn_w), f(attn_b)
    atp_w, atp_b = f(atp_w), f(atp_b)
    fc_w, fc_b, pr_w, pr_b = f(fc_w), f(fc_b), f(pr_w), f(pr_b)

    # fold v-bias through atp: y_true = y/den + bv  ->  + bv @ atp_w
    bv_full = attn_b[:, 2 * E:]                       # [L, 768]
    atpb_eff = atp_b + np.einsum('le,leo->lo', bv_full, atp_w)

    in_maps = []
    for c in range(NC):
        hs = 3 * (c % 4)
        q = [attn_w[:, :, (hs + hh) * HD:(hs + hh + 1) * HD] for hh in range(3)]
        k = [attn_w[:, :, E + (hs + hh) * HD:E + (hs + hh + 1) * HD] for hh in range(3)]
        v = [attn_w[:, :, 2 * E + (hs + hh) * HD:2 * E + (hs + hh + 1) * HD] for hh in range(3)]
        pad = np.zeros((L, E, HD), np.float32)
        # cols: [q0|q1, k0|k1, pad|q2, pad|k2]
        wqk_c = np.concatenate([q[0], q[1], k[0], k[1], pad, q[2], pad, k[2]], axis=2)
        qb = [attn_b[:, (hs + hh) * HD:(hs + hh + 1) * HD] for hh in range(3)]
        kb = [attn_b[:, E + (hs + hh) * HD:E + (hs + hh + 1) * HD] for hh in range(3)]
        zb = np.zeros((L, HD), np.float32)
        bqk_c = np.stack([
            np.concatenate([qb[0], qb[1]], axis=1),
            np.concatenate([kb[0], kb[1]], axis=1),
            np.concatenate([zb, qb[2]], axis=1),
            np.concatenate([zb, kb[2]], axis=1),
        ], axis=2)  # [L, 128, 4]
        wv_c = np.concatenate(v, axis=2)
        in_maps.append({
            "x0s": np.ascontiguousarray(x0[c * TS:(c + 1) * TS]),
            "wqk": h(wqk_c), "bqk": np.ascontiguousarray(bqk_c),
            "wv": h(wv_c),
            "watp": h(atp_w[:, hs * HD:(hs + 3) * HD, :]),
            "atpb": np.ascontiguousarray(atpb_eff),
            "fcw": h(fc_w), "fcb": np.ascontiguousarray(
                fc_b.reshape(L, 24, 128).transpose(0, 2, 1)),
            "prw": h(pr_w), "prb": pr_b,
            "ln1g": f(ln1_w), "ln1b": f(ln1_b),
            "ln2g": f(ln2_w), "ln2b": f(ln2_b),
            "lnfg": f(lnf_w).reshape(1, E), "lnfb": f(lnf_b).reshape(1, E),
            "wteT": h(wteT_full[:, c * VS:(c + 1) * VS]),
        })
    return in_maps


def kernel(trace=False, **inputs):
    if "nc" not in _CACHE:
        _CACHE["nc"] = _build_program()
    nc = _CACHE["nc"]
    in_maps = _prep_inputs(**inputs)
    res = run_bass_kernel_spmd(nc, in_maps, core_ids=list(range(NC)), trace=trace)
    _CACHE["last_result"] = res
    logits = np.concatenate([res.results[c]["logits"] for c in range(NC)], axis=1)
    return logits[:, :V].reshape(B, T, V).astype(np.float32)


# revision 27
# speedup vs baseline: 1.6054x; 1.0169x over previous
"""GPT (4-layer, E=768, H=12, T=1024, B=2, V=50257) forward on 8 trn2 cores.

Sharding:
  - Residual stream x token-sharded fp32: core c owns tokens [c*256,(c+1)*256)
    of the flattened [2048] (batch-major): cores 0-3 = batch 0, 4-7 = batch 1.
  - Attention head-sharded within each batch group of 4 cores (3 heads each):
    AllGather hidden (fp16, split in 2 halves for overlap), compute q/k/v +
    scores + y for my heads over all 1024 tokens, then AllToAll the normalized
    per-head y back to token owners (uniform SPMD), out-proj token-local with
    full 768 contraction (fp32 psum, no low-precision reduction anywhere).
  - MLP fully token-local: fc weight-stationary (out [hid,tok]),
    pr activation-stationary (out [tok,E]) - no transposes inside MLP.
  - lm_head vocab-sharded fp16: AllGather lnf(x) (all 8), each core computes
    [2048, 6284] logit slice, vocab-group-outer loop with resident xfT.
  - All matmul inputs fp16 (fp32 psum accumulate); scores 2-head row-packed
    (tile_position concurrency); h2 on row-group 64-127.
"""

import sys
from contextlib import ExitStack
import numpy as np

sys.path.insert(0, "/opt/trn_rl_repo")

import concourse.bass as bass
import concourse.mybir as mybir
import concourse.tile as tile
from concourse import bacc
from concourse.bass_utils import run_bass_kernel_spmd
from concourse.masks import make_identity

L, H, E, T, V = 4, 12, 768, 1024, 50257
B = 2
NC = 8
TS = (B * T) // NC          # 256 tokens per core
VS = 6284                   # vocab slice per core (padded V = 50272)
VPAD = VS * NC
HD = 64
EPS = 1e-5
SCALE = float(1.0 / np.sqrt(np.float32(E)))
F32 = mybir.dt.float32
F16 = mybir.dt.float16

L_RUN = L  # layers actually executed (tests may truncate)
SIM_GELU = False  # sim lacks Gelu_apprx_tanh; use x*sigmoid(1.702x) for debug
DEBUG_DUMP = False  # dump layer-0 intermediates to a "dbg" output
_CACHE = {}


def _build_program():
    nc = bacc.Bacc("TRN2", target_bir_lowering=False, debug=False, num_devices=NC)

    # ---- I/O -------------------------------------------------------------
    x0s = nc.dram_tensor("x0s", [TS, E], F32, kind="ExternalInput")
    # wqk cols: [q0|q1 (128), k0|k1 (128), pad|q2 (128), pad|k2 (128)]
    wqk = nc.dram_tensor("wqk", [L, E, 512], F16, kind="ExternalInput")
    bqk = nc.dram_tensor("bqk", [L, 128, 4], F32, kind="ExternalInput")
    wv = nc.dram_tensor("wv", [L, E, 3 * HD], F16, kind="ExternalInput")
    watp = nc.dram_tensor("watp", [L, E, E], F16, kind="ExternalInput")  # full (head-major rows)
    atpb = nc.dram_tensor("atpb", [L, E], F32, kind="ExternalInput")     # includes bv@watp fold
    fcw = nc.dram_tensor("fcw", [L, E, 4 * E], F16, kind="ExternalInput")
    fcb = nc.dram_tensor("fcb", [L, 128, 24], F32, kind="ExternalInput")
    prw = nc.dram_tensor("prw", [L, 4 * E, E], F16, kind="ExternalInput")
    prb = nc.dram_tensor("prb", [L, E], F32, kind="ExternalInput")
    ln1g = nc.dram_tensor("ln1g", [L, E], F32, kind="ExternalInput")
    ln1b = nc.dram_tensor("ln1b", [L, E], F32, kind="ExternalInput")
    ln2g = nc.dram_tensor("ln2g", [L, E], F32, kind="ExternalInput")
    ln2b = nc.dram_tensor("ln2b", [L, E], F32, kind="ExternalInput")
    lnfg = nc.dram_tensor("lnfg", [1, E], F32, kind="ExternalInput")
    lnfb = nc.dram_tensor("lnfb", [1, E], F32, kind="ExternalInput")
    wteT = nc.dram_tensor("wteT", [E, VS], F16, kind="ExternalInput")
    logits = nc.dram_tensor("logits", [B * T, VS], F16, kind="ExternalOutput")
    dbg = (nc.dram_tensor("dbg", [12, 128, T], F16, kind="ExternalOutput")
           if DEBUG_DUMP else None)

    tri_np = (np.arange(128)[None, :] >= np.arange(128)[:, None]).astype(np.float16)
    tri = nc.inline_tensor(tri_np, name="tri_const")

    g_all = [list(range(NC))]
    g_batch = [[0, 1, 2, 3], [4, 5, 6, 7]]

    def bcast_row(pool, src_ap, n, dtype, w, tag=None):
        """Replicate a [w] DRAM row across n partitions via broadcast DMA."""
        t = pool.tile([n, w], dtype, tag=tag)
        in_ap = bass.AP(
            tensor=src_ap.tensor,
            offset=src_ap.offset,
            ap=[[0, n]] + [list(p) for p in src_ap.ap],
        )
        eng = nc.gpsimd if dtype != src_ap.dtype else nc.sync
        eng.dma_start(out=t[:], in_=in_ap)
        return t

    with tile.TileContext(nc) as tc, ExitStack() as es:
        const = es.enter_context(tc.tile_pool(name="const", bufs=1))
        xp = es.enter_context(tc.tile_pool(name="xp", bufs=1))
        lnrow = es.enter_context(tc.tile_pool(name="lnrow", bufs=1))
        stat = es.enter_context(tc.tile_pool(name="stat", bufs=2))
        hpool = es.enter_context(tc.tile_pool(name="hpool", bufs=2))
        dram = es.enter_context(tc.tile_pool(name="dram", bufs=1, space="DRAM"))

        ident_f = const.tile([128, 128], F32, name="ident_f")
        make_identity(nc, ident_f)
        ident = const.tile([128, 128], F16, name="ident")
        nc.vector.tensor_copy(ident[:], ident_f[:])
        tri_sb = const.tile([128, 128], F16, name="tri_sb")
        nc.sync.dma_start(out=tri_sb[:], in_=tri[:, :])
        eps_sb = const.tile([128, 1], F32, name="eps_sb")
        nc.vector.memset(eps_sb, EPS)
        ones3 = const.tile([128, 3], F16, name="ones3")
        nc.vector.memset(ones3, 1.0)

        # persistent residual stream [256, 768] fp32 as two [128, 768] tiles
        x_sb = [xp.tile([128, E], F32, tag=f"x{t}", name=f"x{t}") for t in range(2)]
        for t in range(2):
            nc.sync.dma_start(out=x_sb[t][:], in_=x0s[t * 128:(t + 1) * 128, :])

        # DRAM bounce buffers for collectives (fp16)
        hT_in = [dram.tile([3 * 128, TS], F16, name=f"hT_in{i}") for i in range(2)]
        hT_ag = [dram.tile([4 * 3 * 128, TS], F16, name=f"hT_ag{i}") for i in range(2)]
        y_in_y = dram.tile([3 * HD, T], F16, name="y_in_y")
        y_ag = dram.tile([4 * 3 * HD, T], F16, name="y_ag")
        xf_in = [dram.tile([3 * 128, TS], F16, name=f"xf_in{i}") for i in range(2)]
        xf_ag = [dram.tile([NC * 3 * 128, TS], F16, name=f"xf_ag{i}") for i in range(2)]

        def layernorm_t(x_ap, g_bc, b_bc, out_tile):
            """LN over free dim (768) of [128, 768] fp32 tile -> out fp16."""
            stats = stat.tile([128, 3, 6], F32, tag="bn_stats", name="bn_stats_t")
            xr = x_ap.rearrange("p (s d) -> p s d", s=3)
            for s in range(3):
                nc.vector.bn_stats(out=stats[:, s, :], in_=xr[:, s, :])
            mv = stat.tile([128, 2], F32, tag="bn_aggr", name="bn_aggr_t")
            nc.vector.bn_aggr(out=mv[:], in_=stats[:])
            rstd = stat.tile([128, 1], F32, tag="rstd", name="rstd_t")
            nc.scalar.activation(out=rstd[:], in_=mv[:, 1:2],
                                 func=mybir.ActivationFunctionType.Sqrt,
                                 bias=eps_sb[:], scale=1.0)
            nc.vector.reciprocal(out=rstd[:], in_=rstd[:])
            tmp = stat.tile([128, E], F32, tag="ln_tmp", name="ln_tmp")
            nc.vector.tensor_scalar(out=tmp[:], in0=x_ap,
                                    scalar1=mv[:, 0:1], scalar2=rstd[:],
                                    op0=mybir.AluOpType.subtract,
                                    op1=mybir.AluOpType.mult)
            nc.vector.tensor_mul(out=tmp[:], in0=tmp[:], in1=g_bc[:])
            nc.vector.tensor_add(out=out_tile[:], in0=tmp[:], in1=b_bc[:])

        # ---- persistent layer pools (tags reused across layers) ----------
        es_l = es.enter_context(ExitStack())
        wqkp = es_l.enter_context(tc.tile_pool(name="wqkp", bufs=1))
        wvp = es_l.enter_context(tc.tile_pool(name="wvp", bufs=1))
        watpp = es_l.enter_context(tc.tile_pool(name="watpp", bufs=1))
        fcwp = es_l.enter_context(tc.tile_pool(name="fcwp", bufs=1))
        prwp = es_l.enter_context(tc.tile_pool(name="prwp", bufs=1))
        bias_p = es_l.enter_context(tc.tile_pool(name="bias_p", bufs=2))
        hTp = es_l.enter_context(tc.tile_pool(name="hTp", bufs=1))
        hTbp = es_l.enter_context(tc.tile_pool(name="hTbp", bufs=1))
        qkp = es_l.enter_context(tc.tile_pool(name="qkp", bufs=1))
        vp = es_l.enter_context(tc.tile_pool(name="vp", bufs=1))
        ep = es_l.enter_context(tc.tile_pool(name="ep", bufs=1))
        yp = es_l.enter_context(tc.tile_pool(name="yp", bufs=1))
        sm = es_l.enter_context(tc.tile_pool(name="sm", bufs=1))
        mTp = es_l.enter_context(tc.tile_pool(name="mTp", bufs=1))
        yallp = es_l.enter_context(tc.tile_pool(name="yallp", bufs=1))

        # v_sb layout [128, 258]: h0 [v0|1] at 0:65, h1 [1|pad63|v1] at
        # 65:193 (den at psum row 0, y1 at rows 64-127 for packed
        # out-proj), h2 [v2|1] at 193:258. Constant cols written once.
        v_sb = [vp.tile([128, 258], F16, tag=f"v{t}", name=f"v{t}") for t in range(8)]
        for t in range(8):
            nc.vector.tensor_copy(v_sb[t][:, 64:65], ones3[:, 0:1])
            nc.vector.tensor_copy(v_sb[t][:, 65:66], ones3[:, 1:2])
            nc.vector.tensor_copy(v_sb[t][:, 257:258], ones3[:, 2:3])
            nc.vector.memset(v_sb[t][:, 66:129], 0.0)

        for layer in range(L_RUN):
            # ---- weight loads (gpsimd queue; Tile schedules early) ------
            wqk_sb = [wqkp.tile([128, 512], F16, tag=f"wqk{k}", name=f"wqk{k}") for k in range(6)]
            wv_sb = [wvp.tile([128, 192], F16, tag=f"wv{k}", name=f"wv{k}") for k in range(6)]
            watp_sb = [watpp.tile([128, E], F16, tag=f"wa{k}", name=f"wa{k}") for k in range(6)]
            for k in range(6):
                nc.gpsimd.dma_start(out=watp_sb[k][:], in_=watp[layer, k * 128:(k + 1) * 128, :])
                nc.gpsimd.dma_start(out=wqk_sb[k][:], in_=wqk[layer, k * 128:(k + 1) * 128, :])
                nc.gpsimd.dma_start(out=wv_sb[k][:], in_=wv[layer, k * 128:(k + 1) * 128, :])
            fcw_sb = [fcwp.tile([128, 4 * E], F16, tag=f"fcw{k}", name=f"fcw{k}") for k in range(6)]
            for k in range(6):
                nc.gpsimd.dma_start(out=fcw_sb[k][:], in_=fcw[layer, k * 128:(k + 1) * 128, :])
            prw_sb = [prwp.tile([128, E], F16, tag=f"prw{m}", name=f"prw{m}") for m in range(24)]
            for m in range(24):
                nc.gpsimd.dma_start(out=prw_sb[m][:], in_=prw[layer, m * 128:(m + 1) * 128, :])
            bqk_sb = bias_p.tile([128, 4], F32, tag="bqk", name="bqk")
            nc.sync.dma_start(out=bqk_sb[:], in_=bqk[layer])
            fcb_sb = bias_p.tile([128, 24], F32, tag="fcb", name="fcb")
            nc.sync.dma_start(out=fcb_sb[:], in_=fcb[layer])
            ln1g_bc = bcast_row(lnrow, ln1g[layer], 128, F16, E, tag="ln1g")
            ln1b_bc = bcast_row(lnrow, ln1b[layer], 128, F16, E, tag="ln1b")

            # ---- LN1 + transpose -> hT [768, 256] fp16 -------------------
            es_t1 = ExitStack()
            psT = es_t1.enter_context(tc.tile_pool(name="psT", bufs=4, space="PSUM"))
            hT = [hTp.tile([128, TS], F16, tag=f"hT{k}", name=f"hT{k}") for k in range(6)]
            for t in range(2):
                h_t = hpool.tile([128, E], F16, tag="h", name="h")
                layernorm_t(x_sb[t][:], ln1g_bc, ln1b_bc, h_t)
                for k in range(6):
                    pt = psT.tile([128, 128], F16, tag="tr", name="tr")
                    nc.tensor.transpose(pt[:], h_t[:, k * 128:(k + 1) * 128], ident[:])
                    dst = hT[k][:, t * 128:(t + 1) * 128]
                    if k % 2 == 0:
                        nc.vector.tensor_copy(dst, pt[:])
                    else:
                        nc.scalar.activation(out=dst, in_=pt[:],
                                             func=mybir.ActivationFunctionType.Copy)
            es_t1.close()

            # ---- AllGather hidden in 2 halves (within batch group of 4) --
            for half in range(2):
                for k in range(3):
                    nc.sync.dma_start(out=hT_in[half][k * 128:(k + 1) * 128, :],
                                      in_=hT[half * 3 + k][:])
                nc.gpsimd.collective_compute(
                    "AllGather", mybir.AluOpType.bypass,
                    replica_groups=g_batch,
                    ins=[hT_in[half].opt()],
                    outs=[hT_ag[half].opt()],
                )
            # load hTb 6 x [128, 1024] fp16 (one 3D-AP DMA per k-chunk)
            hTb = [hTbp.tile([128, T], F16, tag=f"hTb{k}", name=f"hTb{k}") for k in range(6)]
            for k in range(6):
                half, kk = divmod(k, 3)
                src = hT_ag[half]
                in_ap = bass.AP(
                    tensor=src.tensor,
                    offset=src[kk * 128, 0].offset,
                    ap=[[TS, 128], [3 * 128 * TS, 4], [1, TS]],
                )
                nc.sync.dma_start(
                    out=hTb[k][:].rearrange("p (r t) -> p r t", r=4),
                    in_=in_ap)

            if DEBUG_DUMP and layer == 0:
                nc.sync.dma_start(out=dbg[0], in_=hTb[0][:])
                nc.sync.dma_start(out=dbg[11][:, 0:128], in_=tri_sb[:])

            # ---- QKV ----------------------------------------------------
            es_a = ExitStack()
            psQK = es_a.enter_context(tc.tile_pool(name="psQK", bufs=3, space="PSUM"))
            psV = es_a.enter_context(tc.tile_pool(name="psV", bufs=2, space="PSUM"))
            # qT2/kT2: rows 0-63 head0, 64-127 head1; q1T/k1T rows 64-127 head2
            qT2 = qkp.tile([128, T], F16, tag="qT2", name="qT2")
            kT2 = qkp.tile([128, T], F16, tag="kT2", name="kT2")
            q1T = qkp.tile([128, T], F16, tag="q1T", name="q1T")
            k1T = qkp.tile([128, T], F16, tag="k1T", name="k1T")
            qk_dsts = [qT2, kT2, q1T, k1T]
            for s in range(4):
                ps = psQK.tile([128, T], F32, tag="qk", name="qk")
                for n in range(2):
                    for k in range(6):
                        nc.tensor.matmul(ps[:, n * 512:(n + 1) * 512],
                                         wqk_sb[k][:, s * 128:(s + 1) * 128],
                                         hTb[k][:, n * 512:(n + 1) * 512],
                                         start=(k == 0), stop=(k == 5))
                if s < 2:
                    nc.vector.tensor_scalar_add(out=qk_dsts[s][:], in0=ps[:],
                                                scalar1=bqk_sb[:, s:s + 1])
                else:
                    nc.vector.tensor_scalar_add(out=qk_dsts[s][64:128, :],
                                                in0=ps[64:128, :],
                                                scalar1=bqk_sb[64:128, s:s + 1])
            for t in range(8):
                ps = psV.tile([128, 192], F32, tag="v", name="v")
                for k in range(6):
                    nc.tensor.matmul(ps[:], hTb[k][:, t * 128:(t + 1) * 128],
                                     wv_sb[k][:], start=(k == 0), stop=(k == 5))
                if t % 2 == 0:
                    nc.vector.tensor_copy(v_sb[t][:, 0:64], ps[:, 0:64])
                    nc.vector.tensor_copy(v_sb[t][:, 129:193], ps[:, 64:128])
                    nc.vector.tensor_copy(v_sb[t][:, 193:257], ps[:, 128:192])
                else:
                    nc.scalar.activation(out=v_sb[t][:, 0:64], in_=ps[:, 0:64],
                                         func=mybir.ActivationFunctionType.Copy)
                    nc.scalar.activation(out=v_sb[t][:, 129:193], in_=ps[:, 64:128],
                                         func=mybir.ActivationFunctionType.Copy)
                    nc.scalar.activation(out=v_sb[t][:, 193:257], in_=ps[:, 128:192],
                                         func=mybir.ActivationFunctionType.Copy)

            if DEBUG_DUMP and layer == 0:
                nc.sync.dma_start(out=dbg[1], in_=qT2[:])
                nc.sync.dma_start(out=dbg[2], in_=kT2[:])
                nc.sync.dma_start(out=dbg[3], in_=k1T[:])
                nc.sync.dma_start(out=dbg[10][:, 0:258], in_=v_sb[0][:])

            # ---- attention: scores + y, head 0/1 row-packed --------------
            es_a.close()
            es_b = ExitStack()
            psS = es_b.enter_context(tc.tile_pool(name="psS", bufs=2, space="PSUM"))
            psY = es_b.enter_context(tc.tile_pool(name="psY", bufs=1, space="PSUM"))
            # h0: y rows 0:64, den 64; h1: den 0, y 64:128; h2: y 0:64, den 64
            yps0 = psY.tile([65, T], F32, tag="y0", name="y0")
            yps1 = psY.tile([128, T], F32, tag="y1", name="y1")
            yps2 = psY.tile([65, T], F32, tag="y2", name="y2")
            heads = [(qT2[0:64, :], kT2[0:64, :], yps0, 0, 65),
                     (qT2[64:128, :], kT2[64:128, :], yps1, 65, 193),
                     (q1T[64:128, :], k1T[64:128, :], yps2, 193, 258)]
            for j in range(8):
                qs = j * 128
                qlen = T - qs
                for h in range(3):
                    qT_h, kT_h, yout, v0, v1 = heads[h]
                    e_sb = ep.tile([128, T], F16, tag=f"e{h}", name=f"e{h}")
                    off = 0
                    while off < qlen:
                        cl = min(512, qlen - off)
                        pss = psS.tile([128, 512], F32, tag="s", name="s")
                        nc.tensor.matmul(pss[:, 0:cl], kT_h[:, qs:qs + 128],
                                         qT_h[:, qs + off: qs + off + cl],
                                         start=True, stop=True)
                        nc.scalar.activation(out=e_sb[:, off:off + cl],
                                             in_=pss[:, 0:cl],
                                             func=mybir.ActivationFunctionType.Exp,
                                             scale=SCALE)
                        off += cl
                    nc.vector.tensor_mul(out=e_sb[:, 0:128], in0=e_sb[:, 0:128],
                                         in1=tri_sb[:])
                    if qs < 512:
                        nc.tensor.matmul(yout[:, qs:512],
                                         v_sb[j][:, v0:v1],
                                         e_sb[:, 0:512 - qs],
                                         start=(j == 0), stop=(j == 3))
                    nc.tensor.matmul(yout[:, max(qs, 512):T],
                                     v_sb[j][:, v0:v1],
                                     e_sb[:, max(qs, 512) - qs:qlen],
                                     start=(j == 0), stop=(j == 7))

            # ---- normalize into packed y tiles ---------------------------
            # y01T rows 0:64 = head0, 64:128 = head1; y2T rows 0:64 = head2
            y01T = yp.tile([128, T], F16, tag="y01T", name="y01T")
            y2T = yp.tile([64, T], F16, tag="y2T", name="y2T")
            for h, (yout, den_row, dst) in enumerate(
                    [(yps0, 64, y01T[0:64, :]), (yps1, 0, y01T[64:128, :]),
                     (yps2, 64, y2T[:])]):
                recip = sm.tile([1, T], F32, tag="recip", name="recip")
                nc.vector.reciprocal(out=recip[:], in_=yout[den_row:den_row + 1, :])
                # broadcast to all 128 partitions (HW partition_broadcast
                # mishandles a base-64 target), then use the aligned slice
                bcf = sm.tile([128, T], F32, tag="bcf", name="bcf")
                nc.gpsimd.partition_broadcast(bcf[:], recip[:])
                lo = 64 if h == 1 else 0
                ysrc = yout[64:128, :] if h == 1 else yout[0:64, :]
                nc.vector.tensor_mul(out=dst, in0=ysrc, in1=bcf[lo:lo + 64, :])
            if DEBUG_DUMP and layer == 0:
                nc.sync.dma_start(out=dbg[4], in_=y01T[:])
                nc.sync.dma_start(out=dbg[5][0:64, :], in_=y2T[:])
            es_b.close()

            # ---- AllGather y (all heads, all tokens) ---------------------
            nc.sync.dma_start(out=y_in_y[0:128, :], in_=y01T[:])
            nc.sync.dma_start(out=y_in_y[128:192, :], in_=y2T[:])
            nc.gpsimd.collective_compute(
                "AllGather", mybir.AluOpType.bypass,
                replica_groups=g_batch,
                ins=[y_in_y.opt()],
                outs=[y_ag.opt()],
            )
            # own-token slice [768, 256] via rank-dependent column offset
            r4 = nc.gpsimd.partition_id() % 4
            yall = [yallp.tile([128, TS], F16, tag=f"ya{k}", name=f"ya{k}") for k in range(6)]
            for k in range(6):
                in_ap = bass.AP(
                    tensor=y_ag.tensor,
                    offset=r4 * TS + y_ag[k * 128, 0].offset,
                    ap=[[T, 128], [1, TS]],
                )
                nc.gpsimd.dma_start(out=yall[k][:], in_=in_ap)

            # ---- out-proj (own 256 tokens, full 768 contraction) ---------
            es_c = ExitStack()
            psO = es_c.enter_context(tc.tile_pool(name="psO", bufs=2, space="PSUM"))
            atpb_bc = bcast_row(lnrow, atpb[layer], 128, F16, E, tag="atpb")
            for t in range(2):
                po = psO.tile([128, E], F32, tag="o", name="o")
                for n0, n1 in ((0, 512), (512, 768)):
                    for k in range(6):
                        nc.tensor.matmul(po[:, n0:n1],
                                         yall[k][:, t * 128:(t + 1) * 128],
                                         watp_sb[k][:, n0:n1],
                                         start=(k == 0), stop=(k == 5))
                nc.vector.tensor_add(out=x_sb[t][:], in0=x_sb[t][:], in1=po[:])
                nc.vector.tensor_add(out=x_sb[t][:], in0=x_sb[t][:], in1=atpb_bc[:])
            if DEBUG_DUMP and layer == 0:
                for t in range(2):
                    xc = hpool.tile([128, E], F16, tag="ao", name="xc")
                    nc.vector.tensor_copy(xc[:], x_sb[t][:])
                    nc.sync.dma_start(out=dbg[8 + t][:, 0:E], in_=xc[:])
            es_c.close()

            # ---- LN2 + transpose -> h2T ---------------------------------
            ln2g_bc = bcast_row(lnrow, ln2g[layer], 128, F16, E, tag="ln2g")
            ln2b_bc = bcast_row(lnrow, ln2b[layer], 128, F16, E, tag="ln2b")
            es_t2 = ExitStack()
            psT2 = es_t2.enter_context(tc.tile_pool(name="psT2", bufs=4, space="PSUM"))
            h2T = [hTp.tile([128, TS], F16, tag=f"h2T{k}", name=f"h2T{k}") for k in range(6)]
            for t in range(2):
                h_t = hpool.tile([128, E], F16, tag="h", name="h")
                layernorm_t(x_sb[t][:], ln2g_bc, ln2b_bc, h_t)
                for k in range(6):
                    pt = psT2.tile([128, 128], F16, tag="tr2", name="tr2")
                    nc.tensor.transpose(pt[:], h_t[:, k * 128:(k + 1) * 128], ident[:])
                    dst = h2T[k][:, t * 128:(t + 1) * 128]
                    if k % 2 == 0:
                        nc.vector.tensor_copy(dst, pt[:])
                    else:
                        nc.scalar.activation(out=dst, in_=pt[:],
                                             func=mybir.ActivationFunctionType.Copy)
            es_t2.close()

            # ---- MLP fc: mT[m] = gelu(fcw[:,m].T @ h2T + fcb[m]) ---------
            es_d = ExitStack()
            psM = es_d.enter_context(tc.tile_pool(name="psM", bufs=4, space="PSUM"))
            psP = es_d.enter_context(tc.tile_pool(name="psP", bufs=1, space="PSUM"))
            mT = [mTp.tile([128, TS], F16, tag=f"mT{m}", name=f"mT{m}") for m in range(24)]
            if SIM_GELU:
                fcb17 = bias_p.tile([128, 24], F32, tag="fcb17", name="fcb17")
                nc.vector.tensor_scalar(out=fcb17[:], in0=fcb_sb[:], scalar1=1.702,
                                        scalar2=None,
                                        op0=mybir.AluOpType.mult)
            for m in range(24):
                ps = psM.tile([128, TS], F32, tag="m", name="m")
                for k in range(6):
                    nc.tensor.matmul(ps[:], fcw_sb[k][:, m * 128:(m + 1) * 128],
                                     h2T[k][:], start=(k == 0), stop=(k == 5))
                if SIM_GELU:
                    sgm = mTp.tile([128, TS], F32, tag="sgm", name="sgm")
                    nc.scalar.activation(out=sgm[:], in_=ps[:],
                                         func=mybir.ActivationFunctionType.Sigmoid,
                                         scale=1.702, bias=fcb17[:, m:m + 1])
                    nc.vector.tensor_scalar_add(out=mT[m][:], in0=ps[:],
                                                scalar1=fcb_sb[:, m:m + 1])
                    nc.vector.tensor_mul(out=mT[m][:], in0=mT[m][:], in1=sgm[:])
                else:
                    nc.scalar.activation(out=mT[m][:], in_=ps[:],
                                         func=mybir.ActivationFunctionType.Gelu_apprx_tanh,
                                         bias=fcb_sb[:, m:m + 1])

            # ---- MLP pr + residual --------------------------------------
            prb_bc = bcast_row(lnrow, prb[layer], 128, F16, E, tag="prb")
            ps2 = [psP.tile([128, E], F32, tag=f"p{t}", name=f"p{t}") for t in range(2)]
            for m in range(24):
                for t in range(2):
                    for n0, n1 in ((0, 512), (512, 768)):
                        nc.tensor.matmul(ps2[t][:, n0:n1],
                                         mT[m][:, t * 128:(t + 1) * 128],
                                         prw_sb[m][:, n0:n1],
                                         start=(m == 0), stop=(m == 23))
            for t in range(2):
                nc.vector.tensor_add(out=x_sb[t][:], in0=x_sb[t][:], in1=ps2[t][:])
                nc.vector.tensor_add(out=x_sb[t][:], in0=x_sb[t][:], in1=prb_bc[:])
            es_d.close()

        # ---- final LN + AllGather(all 8) + lm_head -----------------------
        lnfg_bc = bcast_row(lnrow, lnfg[0], 128, F16, E, tag="lnfg")
        lnfb_bc = bcast_row(lnrow, lnfb[0], 128, F16, E, tag="lnfb")
        es_tf = ExitStack()
        psTf = es_tf.enter_context(tc.tile_pool(name="psTf", bufs=4, space="PSUM"))
        xfT = [hTp.tile([128, TS], F16, tag=f"hT{k}", name=f"xfT{k}") for k in range(6)]
        for t in range(2):
            h_t = hpool.tile([128, E], F16, tag="h", name="h")
            layernorm_t(x_sb[t][:], lnfg_bc, lnfb_bc, h_t)
            for k in range(6):
                pt = psTf.tile([128, 128], F16, tag="trf", name="trf")
                nc.tensor.transpose(pt[:], h_t[:, k * 128:(k + 1) * 128], ident[:])
                dst = xfT[k][:, t * 128:(t + 1) * 128]
                if k % 2 == 0:
                    nc.vector.tensor_copy(dst, pt[:])
                else:
                    nc.scalar.activation(out=dst, in_=pt[:],
                                         func=mybir.ActivationFunctionType.Copy)
        es_tf.close()
        for half in range(2):
            for k in range(3):
                nc.sync.dma_start(out=xf_in[half][k * 128:(k + 1) * 128, :],
                                  in_=xfT[half * 3 + k][:])
            nc.gpsimd.collective_compute(
                "AllGather", mybir.AluOpType.bypass,
                replica_groups=g_all,
                ins=[xf_in[half].opt()],
                outs=[xf_ag[half].opt()],
            )
        es_l.close()
        es_h = es.enter_context(ExitStack())
        xfp = es_h.enter_context(tc.tile_pool(name="xfp", bufs=1))
        wtep = es_h.enter_context(tc.tile_pool(name="wtep", bufs=2))
        lop = es_h.enter_context(tc.tile_pool(name="lop", bufs=4))
        psL = es_h.enter_context(tc.tile_pool(name="psL", bufs=1, space="PSUM"))

        # xfT_full 6 x [128, 2048] fp16 (3D-AP load across 8 ranks)
        xf_full = [xfp.tile([128, B * T], F16, tag=f"xf{k}", name=f"xf{k}") for k in range(6)]
        for k in range(6):
            half, kk = divmod(k, 3)
            src_t = xf_ag[half]
            in_ap = bass.AP(
                tensor=src_t.tensor,
                offset=src_t[kk * 128, 0].offset,
                ap=[[TS, 128], [3 * 128 * TS, NC], [1, TS]],
            )
            nc.sync.dma_start(out=xf_full[k][:].rearrange("p (r t) -> p r t", r=NC),
                              in_=in_ap)

        nch = (VS + 511) // 512
        for n in range(nch):
            n0 = n * 512
            nw = min(512, VS - n0)
            wte_sb = [wtep.tile([128, 512], F16, tag=f"wte{k}", name=f"wte{k}") for k in range(6)]
            for k in range(6):
                nc.gpsimd.dma_start(out=wte_sb[k][:, 0:nw],
                                    in_=wteT[k * 128:(k + 1) * 128, n0:n0 + nw])
            for th in range(2):
                pss = [psL.tile([128, 512], F32, tag=f"l{t}", name=f"l{t}") for t in range(8)]
                for k in range(6):
                    for t in range(8):
                        nc.tensor.matmul(pss[t][:, 0:nw],
                                         xf_full[k][:, (th * 8 + t) * 128:(th * 8 + t + 1) * 128],
                                         wte_sb[k][:, 0:nw],
                                         start=(k == 0), stop=(k == 5))
                for t in range(8):
                    lo = lop.tile([128, 512], F16, tag="lo", name="lo")
                    if t % 2 == 0:
                        nc.vector.tensor_copy(lo[:, 0:nw], pss[t][:, 0:nw])
                    else:
                        nc.scalar.activation(out=lo[:, 0:nw], in_=pss[t][:, 0:nw],
                                             func=mybir.ActivationFunctionType.Copy)
                    nc.sync.dma_start(
                        out=logits[(th * 8 + t) * 128:(th * 8 + t + 1) * 128, n0:n0 + nw],
                        in_=lo[:, 0:nw])

    nc.compile()
    return nc


def _prep_inputs(idx, wte, wpe, ln1_w, ln1_b, attn_w, attn_b, atp_w, atp_b,
                 ln2_w, ln2_b, fc_w, fc_b, pr_w, pr_b, lnf_w, lnf_b):
    idx = np.asarray(idx)
    f = lambda a: np.ascontiguousarray(np.asarray(a), dtype=np.float32)
    h = lambda a: np.ascontiguousarray(np.asarray(a), dtype=np.float16)
    wte, wpe = f(wte), f(wpe)
    x0 = wte[idx.reshape(-1)] + np.tile(wpe[:T], (B, 1))  # [2048, 768]
    wte_pad = np.zeros((VPAD, E), np.float32)
    wte_pad[:V] = wte
    wteT_full = np.ascontiguousarray(wte_pad.T)  # [768, VPAD]

    attn_w, attn_b = f(attn_w), f(attn_b)
    atp_w, atp_b = f(atp_w), f(atp_b)
    fc_w, fc_b, pr_w, pr_b = f(fc_w), f(fc_b), f(pr_w), f(pr_b)

    # fold v-bias through atp: y_true = y/den + bv  ->  + bv @ atp_w
    bv_full = attn_b[:, 2 * E:]                       # [L, 768]
    atpb_eff = atp_b + np.einsum('le,leo->lo', bv_full, atp_w)

    in_maps = []
    for c in range(NC):
        hs = 3 * (c % 4)
        q = [attn_w[:, :, (hs + hh) * HD:(hs + hh + 1) * HD] for hh in range(3)]
        k = [attn_w[:, :, E + (hs + hh) * HD:E + (hs + hh + 1) * HD] for hh in range(3)]
        v = [attn_w[:, :, 2 * E + (hs + hh) * HD:2 * E + (hs + hh + 1) * HD] for hh in range(3)]
        pad = np.zeros((L, E, HD), np.float32)
        # cols: [q0|q1, k0|k1, pad|q2, pad|k2]
        wqk_c = np.concatenate([q[0], q[1], k[0], k[1], pad, q[2], pad, k[2]], axis=2)
        qb = [attn_b[:, (hs + hh) * HD:(hs + hh + 1) * HD] for hh in range(3)]
        kb = [attn_b[:, E + (hs + hh) * HD:E + (hs + hh + 1) * HD] for hh in range(3)]
        zb = np.zeros((L, HD), np.float32)
        bqk_c = np.stack([
            np.concatenate([qb[0], qb[1]], axis=1),
            np.concatenate([kb[0], kb[1]], axis=1),
            np.concatenate([zb, qb[2]], axis=1),
            np.concatenate([zb, kb[2]], axis=1),
        ], axis=2)  # [L, 128, 4]
        wv_c = np.concatenate(v, axis=2)
        in_maps.append({
            "x0s": np.ascontiguousarray(x0[c * TS:(c + 1) * TS]),
            "wqk": h(wqk_c), "bqk": np.ascontiguousarray(bqk_c),
            "wv": h(wv_c),
            "watp": h(atp_w),
            "atpb": np.ascontiguousarray(atpb_eff),
            "fcw": h(fc_w), "fcb": np.ascontiguousarray(
                fc_b.reshape(L, 24, 128).transpose(0, 2, 1)),
            "prw": h(pr_w), "prb": pr_b,
            "ln1g": f(ln1_w), "ln1b": f(ln1_b),
            "ln2g": f(ln2_w), "ln2b": f(ln2_b),
            "lnfg": f(lnf_w).reshape(1, E), "lnfb": f(lnf_b).reshape(1, E),
            "wteT": h(wteT_full[:, c * VS:(c + 1) * VS]),
        })
    return in_maps


def kernel(trace=False, **inputs):
    if "nc" not in _CACHE:
        _CACHE["nc"] = _build_program()
    nc = _CACHE["nc"]
    in_maps = _prep_inputs(**inputs)
    res = run_bass_kernel_spmd(nc, in_maps, core_ids=list(range(NC)), trace=trace)
    _CACHE["last_result"] = res
    logits = np.concatenate([res.results[c]["logits"] for c in range(NC)], axis=1)
    return logits[:, :V].reshape(B, T, V).astype(np.float32)
